# revision 8
# baseline (speedup 1.0000x reference)
"""CRF negative log-likelihood on 8 Trainium2 NeuronCores.

Strategy
--------
Pure data-parallel over batch: B=256 -> 32 sequences per core.

Denominator (log-partition) in linear probability domain with
TIME-SEGMENTED chains: each direction (fwd from t=0, bwd from t=S-1)
is split into P=30 overlapping segments run as independent chains.
Segment k>0 starts L=3 steps early from a uniform vector; after the
overlap its state is proportional to the true forward/backward vector
(positive-matrix contraction), and the unknown scales cancel exactly
through stitch ratios measured where adjacent segments share a
timestep.  This cuts the sequential depth from 1024 rounds to n=38.

Per round the 30 fwd chains + 30 bwd chains are stacked into two
[96, 480] streams (fwd tags on partitions 0:47, bwd on 48:95), each:
one stationary block-diag(W, W^T) matmul + one DVE multiply by
g=exp(em-C).  The two streams phase-interleave so PE/DVE overlap.

Numerator: emissions gathered by tag on host (pure indexing) and
summed on device; transition/start/end scores via host-built integer
count matrices contracted against the parameter vector on device.

Chain data bf16; PSUM accumulation f32; logs deferred to one tail.
mask is all-ones per the problem spec and is not consumed.
"""

import os
import sys

import numpy as np

sys.path.insert(0, "/opt/trn_rl_repo")

from contextlib import ExitStack

import ml_dtypes

import concourse.bass as bass
import concourse.tile as tile
from concourse import bacc, mybir
from concourse.bass_utils import run_bass_kernel_spmd

F32 = mybir.dt.float32
BF16 = mybir.dt.bfloat16
AF = mybir.ActivationFunctionType
ALU = mybir.AluOpType

B, S, T = 256, 2048, 48
NCORES = 8
BS = B // NCORES            # 32 sequences per core
TT = 2 * T                  # stacked state size (96)
C_PRE = 4.4                 # constant pre-scale inside exp (keeps p ~O(1))

# segmented-chain geometry (per direction): P segments, stride SEG_S,
# chain length N_R rounds, warmup L_W = N_R - 1 - SEG_S
P_SEG = 30
SEG_S = 34
N_R = (S // 2) - (P_SEG - 1) * SEG_S     # 38 rounds
L_W = N_R - 1 - SEG_S                    # 3 warmup rounds
assert N_R + (P_SEG - 1) * (N_R - 1 - L_W) == S // 2
WID = P_SEG * BS                         # 960 columns per round
HW_ = WID // 2                           # 480 per stream
N_KC = 19                                # count-matrix K chunks (19*128 >= 2400)
# round chunks for DMA+exp staging (small early chunks so the chain and
# the round-L_W stash are not stuck behind a big ACT op)
RCHUNKS = [1, 1, 2, 4, 4, 4, 4, 4, 4, 4, 4, 2]
assert sum(RCHUNKS) == N_R

LAST_RESULTS = None         # set by kernel(); test harness reads exec_time_ns


def _build_module():
    nc = bacc.Bacc(
        "TRN2",
        target_bir_lowering=False,
        debug=False,
        enable_asserts=False,
        num_devices=NCORES,
    )
    emp_d = nc.dram_tensor("emp", [TT, N_R * WID], BF16, kind="ExternalInput")
    ge_d = nc.dram_tensor("ge", [BS, S], F32, kind="ExternalInput")
    bdw_d = nc.dram_tensor("bdw", [TT, TT], F32, kind="ExternalInput")
    trT_d = nc.dram_tensor("trT", [T, T], F32, kind="ExternalInput")
    ser_d = nc.dram_tensor("ser", [TT, BS], F32, kind="ExternalInput")
    csm_d = nc.dram_tensor("csm", [TT, 2], F32, kind="ExternalInput")
    cm_d = nc.dram_tensor("cm", [128, N_KC * BS], F32, kind="ExternalInput")
    tp_d = nc.dram_tensor("tp", [128, N_KC], F32, kind="ExternalInput")
    eye_d = nc.dram_tensor("eye", [BS, BS], F32, kind="ExternalInput")
    on2_d = nc.dram_tensor("on2", [2, 1], F32, kind="ExternalInput")
    on48_d = nc.dram_tensor("on48", [T, 1], F32, kind="ExternalInput")
    res_d = nc.dram_tensor("res", [1, BS], F32, kind="ExternalOutput")

    with tile.TileContext(nc) as tc:
        with ExitStack() as ctx:
            _body(ctx, tc, emp_d, ge_d, bdw_d, trT_d, ser_d, csm_d,
                  cm_d, tp_d, eye_d, on2_d, on48_d, res_d)
    _optimize_module(nc)
    nc.compile()
    return nc


def _ldw_sig(i):
    ap = i.ins[0]
    return (ap.memref, str(ap.ap), ap.offset)


def _optimize_module(nc):
    """Post-schedule IR cleanup.

    1. Drop LDWEIGHTS whose stationary operand is already loaded (the
       chain re-loads the same block-diag weights every round; the PE
       array keeps them between matmuls).
    2. Drop semaphore waits implied by queue program order: a wait on
       producer P is redundant if this queue already waited on P (or a
       later instruction of P's engine).  Engine queues are FIFO; DMA
       completions are unordered so DMA deps only dedup by exact name.
    """
    for f in nc.m.functions:
        for b in f.blocks:
            insts = b.instructions
            last = None
            removed = []
            for i in list(insts):
                nm = type(i).__name__
                if nm == 'InstLdweights':
                    s = _ldw_sig(i)
                    if s == last:
                        removed.append(i)
                    last = s
            for i in removed:
                insts.remove(i)

            insts = b.instructions
            pos = {}
            cnt = {}
            isdma = {}
            for i in insts:
                e = str(i.engine)
                pos[i.name] = (e, cnt.get(e, 0))
                cnt[e] = cnt.get(e, 0) + 1
                isdma[i.name] = type(i).__name__ == 'InstDMACopy'
            waited = {}
            dma_waited = {}
            for i in insts:
                e = str(i.engine)
                deps = [d for d in i.sync_dependency_names()]
                deps.sort(key=lambda d: pos.get(d, (None, -1))[1], reverse=True)
                for d in deps:
                    if d not in pos:
                        continue
                    pe_, pp_ = pos[d]
                    if isdma[d]:
                        s = dma_waited.setdefault(e, set())
                        if d in s:
                            i.try_remove_dependency(d)
                        else:
                            s.add(d)
                        continue
                    if pe_ == e:
                        i.try_remove_dependency(d)
                        continue
                    if pp_ <= waited.get((e, pe_), -1):
                        i.try_remove_dependency(d)
                    else:
                        waited[(e, pe_)] = pp_


def _body(ctx, tc, emp_d, ge_d, bdw_d, trT_d, ser_d, csm_d,
          cm_d, tp_d, eye_d, on2_d, on48_d, res_d):
    nc = tc.nc
    const = ctx.enter_context(tc.tile_pool(name="const", bufs=1))
    io = ctx.enter_context(tc.tile_pool(name="io", bufs=2))
    ppa = ctx.enter_context(tc.tile_pool(name="ppa", bufs=2))
    ppb = ctx.enter_context(tc.tile_pool(name="ppb", bufs=2))
    tl = ctx.enter_context(tc.tile_pool(name="tl", bufs=2))
    psa = ctx.enter_context(tc.tile_pool(name="psa", bufs=2, space="PSUM"))
    psb = ctx.enter_context(tc.tile_pool(name="psb", bufs=2, space="PSUM"))
    psf = ctx.enter_context(tc.tile_pool(name="psf", bufs=1, space="PSUM"))
    psx = ctx.enter_context(tc.tile_pool(name="psx", bufs=2, space="PSUM"))

    # ---- first round-chunk's DMA goes out before everything else ----
    g_big = const.tile([TT, N_R * WID], BF16, tag="gbig")
    em_t0 = io.tile([TT, RCHUNKS[0] * WID], BF16, tag="em")
    nc.sync.dma_start(em_t0[:], emp_d.ap()[:, : RCHUNKS[0] * WID])

    # ---- constants / parameters ----
    bdw_raw = const.tile([TT, TT], F32, tag="bdwraw")
    nc.sync.dma_start(bdw_raw[:], bdw_d.ap())
    bdw = const.tile([TT, TT], BF16, tag="bdw")
    nc.scalar.activation(bdw[:], bdw_raw[:], AF.Exp)

    negc = const.tile([TT, 1], F32, tag="negc")
    nc.gpsimd.memset(negc[:], -C_PRE)

    # exp of chunk 0 early so round 0/1 start fast
    nc.scalar.activation(g_big[:, : RCHUNKS[0] * WID], em_t0[:], AF.Exp,
                         bias=negc[:])

    trT_raw = const.tile([T, T], F32, tag="trTraw")
    nc.sync.dma_start(trT_raw[:], trT_d.ap())
    wt_lo = const.tile([T, T], BF16, tag="wtlo")
    nc.scalar.activation(wt_lo[:], trT_raw[:], AF.Exp)

    se_raw = const.tile([TT, BS], F32, tag="seraw")
    nc.sync.dma_start(se_raw[:], ser_d.ap())

    cs_raw = const.tile([TT, 2], F32, tag="csraw")
    nc.sync.dma_start(cs_raw[:], csm_d.ap())
    cs_m = const.tile([TT, 2], BF16, tag="csm")
    nc.vector.tensor_copy(cs_m[:], cs_raw[:])

    eye_sb = const.tile([BS, BS], F32, tag="eye")
    nc.sync.dma_start(eye_sb[:], eye_d.ap())
    cm_sb = const.tile([128, N_KC, BS], F32, tag="cm")
    nc.sync.dma_start(cm_sb[:], cm_d.ap().rearrange("p (k b) -> p k b", b=BS))
    tp_sb = const.tile([128, N_KC], F32, tag="tp")
    nc.sync.dma_start(tp_sb[:], tp_d.ap())
    on2_sb = const.tile([2, 1], F32, tag="on2")
    nc.sync.dma_start(on2_sb[:], on2_d.ap())
    on48_sb = const.tile([T, 1], F32, tag="on48")
    nc.sync.dma_start(on48_sb[:], on48_d.ap())

    # numerator emission part: gathered values summed on device (Pool,
    # far off the critical path)
    ge_sb = const.tile([BS, S], F32, tag="ge")
    nc.sync.dma_start(ge_sb[:], ge_d.ap())
    ge_sum = const.tile([BS, 1], F32, tag="gesum")
    ge_scr = const.tile([BS, S], BF16, tag="gescr")
    nc.scalar.activation(ge_scr[:], ge_sb[:], AF.Copy, accum_out=ge_sum[:])

    # ---- init vector B: ones, exp(start)/exp(end) on the exact chains ----
    binit = const.tile([TT, WID], BF16, tag="binit")
    nc.gpsimd.memset(binit[:], 1.0)
    nc.scalar.activation(binit[:, 0:BS], se_raw[:], AF.Exp)

    # ---- round 0: p0 = binit * g0 ----
    pa = ppa.tile([TT, HW_], BF16, tag="pa")
    nc.vector.tensor_tensor(pa[:], binit[:, 0:HW_], g_big[:, 0:HW_], ALU.mult)
    pb = ppb.tile([TT, HW_], BF16, tag="pb")
    nc.vector.tensor_tensor(pb[:], binit[:, HW_:WID], g_big[:, HW_:WID],
                            ALU.mult)

    stash_a = const.tile([TT, HW_], BF16, tag="stasha")
    stash_b = const.tile([TT, HW_], BF16, tag="stashb")

    # ---- main chain rounds ----
    rounds_done = RCHUNKS[0]
    chunk_idx = 1
    for r in range(1, N_R):
        # stage the DMA+exp of the next chunk as soon as its round window
        # nears; io bufs=2 pipelines two chunks ahead
        while rounds_done < N_R and rounds_done <= r + 1:
            lc = RCHUNKS[chunk_idx]
            em_t = io.tile([TT, lc * WID], BF16, tag="em")
            nc.sync.dma_start(
                em_t[:], emp_d.ap()[:, rounds_done * WID:(rounds_done + lc) * WID])
            nc.scalar.activation(
                g_big[:, rounds_done * WID:(rounds_done + lc) * WID],
                em_t[:], AF.Exp, bias=negc[:])
            rounds_done += lc
            chunk_idx += 1

        ga = g_big[:, r * WID: r * WID + HW_]
        gb = g_big[:, r * WID + HW_:(r + 1) * WID]

        mma = psa.tile([TT, HW_], F32, tag="mma")
        nc.tensor.matmul(mma[:], bdw[:], pa[:], start=True, stop=True)
        pa = ppa.tile([TT, HW_], BF16, tag="pa")
        nc.vector.tensor_tensor(pa[:], mma[:], ga, ALU.mult)

        mmb = psb.tile([TT, HW_], F32, tag="mmb")
        nc.tensor.matmul(mmb[:], bdw[:], pb[:], start=True, stop=True)
        pb = ppb.tile([TT, HW_], BF16, tag="pb")
        nc.vector.tensor_tensor(pb[:], mmb[:], gb, ALU.mult)

        if r == L_W:
            # stitch snapshot: states at the shared timestep (ACT copies;
            # ACT only has tiny ops queued here by chunk construction)
            nc.scalar.copy(stash_a[:], pa[:])
            nc.scalar.copy(stash_b[:], pb[:])

    # ---- tail: stitch ratios, junction, numerator, output ----
    # column sums (fwd rows 0:48 -> row 0, bwd rows 48:96 -> row 1)
    zfa = psf.tile([2, HW_], F32, tag="zfa")
    nc.tensor.matmul(zfa[:], cs_m[:], pa[:], start=True, stop=True)
    zfb = psf.tile([2, HW_], F32, tag="zfb")
    nc.tensor.matmul(zfb[:], cs_m[:], pb[:], start=True, stop=True)
    lnfin = tl.tile([2, WID], F32, tag="lnfin")
    nc.scalar.activation(lnfin[:, 0:HW_], zfa[:], AF.Ln)
    nc.scalar.activation(lnfin[:, HW_:WID], zfb[:], AF.Ln)

    zsa = psf.tile([2, HW_], F32, tag="zfa")
    nc.tensor.matmul(zsa[:], cs_m[:], stash_a[:], start=True, stop=True)
    zsb = psf.tile([2, HW_], F32, tag="zfb")
    nc.tensor.matmul(zsb[:], cs_m[:], stash_b[:], start=True, stop=True)
    lnsta = tl.tile([2, WID], F32, tag="lnsta")
    nc.scalar.activation(lnsta[:, 0:HW_], zsa[:], AF.Ln)
    nc.scalar.activation(lnsta[:, HW_:WID], zsb[:], AF.Ln)

    # ln r_k = ln sum(X_k final) - ln sum(X_{k+1} at stash), k = 0..P-2
    nw = (P_SEG - 1) * BS
    lnr = tl.tile([2, nw], F32, tag="lnr")
    nc.vector.tensor_tensor(lnr[:], lnfin[:, 0:nw], lnsta[:, BS:WID],
                            ALU.subtract)
    lnr2 = tl.tile([2, BS], F32, tag="lnr2")
    nc.vector.tensor_reduce(
        lnr2[:], lnr[:].rearrange("p (k b) -> p b k", b=BS),
        axis=mybir.AxisListType.X, op=ALU.add)
    lnrs = psx.tile([1, BS], F32, tag="aux")
    nc.tensor.matmul(lnrs[:], on2_sb[:], lnr2[:], start=True, stop=True)

    # junction: Z = sum_j alpha[j] * (W beta)[j] at the middle
    v_lo = tl.tile([T, BS], BF16, tag="vlo")
    nc.sync.dma_start(v_lo[:], pb[T:TT, HW_ - BS:HW_])
    b_ps = psa.tile([T, BS], F32, tag="mma")
    nc.tensor.matmul(b_ps[:], wt_lo[:], v_lo[:], start=True, stop=True)
    zdot = tl.tile([T, BS], F32, tag="zdot")
    nc.vector.tensor_tensor(zdot[:], b_ps[:], pb[0:T, HW_ - BS:HW_], ALU.mult)
    zj_ps = psb.tile([1, BS], F32, tag="mmb")
    nc.tensor.matmul(zj_ps[:], on48_sb[:], zdot[:], start=True, stop=True)

    den = tl.tile([1, BS], F32, tag="den")
    nc.scalar.activation(den[:], zj_ps[:], AF.Ln)
    nc.vector.tensor_scalar_add(den[:], den[:], float(S * C_PRE))
    nc.vector.tensor_tensor(den[:], den[:], lnrs[:], ALU.add)

    # numerator: count-matrix part + gathered-emission part
    num_ps = psx.tile([BS, 1], F32, tag="aux")
    for k in range(N_KC):
        nc.tensor.matmul(
            num_ps[:], cm_sb[:, k, :], tp_sb[:, k:k + 1],
            start=(k == 0), stop=(k == N_KC - 1),
        )
    num_sb = tl.tile([BS, 1], F32, tag="num")
    nc.vector.tensor_tensor(num_sb[:], num_ps[:], ge_sum[:], ALU.add)
    numt_ps = psx.tile([1, BS], F32, tag="aux")
    nc.tensor.transpose(numt_ps[:], num_sb[:], eye_sb[:])

    resu = tl.tile([1, BS], F32, tag="res")
    nc.vector.tensor_tensor(resu[:], den[:], numt_ps[:], ALU.subtract)
    nc.sync.dma_start(res_d.ap(), resu[:])


_MODULE = None


def _get_module():
    global _MODULE
    if _MODULE is None:
        _MODULE = _build_module()
    return _MODULE


def _marshal(emissions, tags, transitions, start_transitions, end_transitions):
    """Host-side layout marshalling -> list of per-core input dicts.

    Host does indexing/layout only; all float math on tensor data runs
    on device.
    """
    em = np.ascontiguousarray(np.asarray(emissions, dtype=np.float32))
    tg = np.asarray(tags).astype(np.int64)
    tr = np.asarray(transitions, dtype=np.float32)
    st = np.asarray(start_transitions, dtype=np.float32)
    en = np.asarray(end_transitions, dtype=np.float32)

    # segmented paired emission layout: [96, round, chain, seq] per core
    offs = np.arange(P_SEG, dtype=np.int64) * SEG_S           # [P]
    tf = offs[None, :] + np.arange(N_R, dtype=np.int64)[:, None]   # [n, P]
    tb = (S - 1) - tf

    emps = []
    ges = []
    for c in range(NCORES):
        emc = em[c * BS:(c + 1) * BS]                    # [32, S, 48]
        f = emc[:, tf, :]                                # [32, n, P, 48]
        bk = emc[:, tb, :]
        f_t = f.transpose(3, 1, 2, 0)                    # [48, n, P, 32]
        b_t = bk.transpose(3, 1, 2, 0)
        emp = np.concatenate([f_t, b_t], axis=0)         # [96, n, P, 32]
        emp = np.ascontiguousarray(emp).reshape(TT, N_R * WID)
        emps.append(emp.astype(ml_dtypes.bfloat16))
        tgc = tg[c * BS:(c + 1) * BS]
        gec = np.take_along_axis(emc, tgc[:, :, None], axis=2)[:, :, 0]
        ges.append(np.ascontiguousarray(gec.astype(np.float32)))

    # block-diag raw weights: exp() on device gives [W 0; 0 W^T]
    bdw = np.full((TT, TT), -1e30, np.float32)
    bdw[:T, :T] = tr
    bdw[T:, T:] = tr.T
    trT = np.ascontiguousarray(tr.T)
    ser_rep = np.ascontiguousarray(
        np.repeat(np.concatenate([st, en])[:, None], BS, axis=1))
    csm = np.zeros((TT, 2), np.float32)
    csm[:T, 0] = 1.0
    csm[T:, 1] = 1.0

    # count matrices (transitions + start/end indicators) per core
    nent = N_KC * 128
    vals = np.zeros(nent, np.float32)
    vals[: T * T] = tr.reshape(-1)
    vals[T * T: T * T + T] = st
    vals[T * T + T: T * T + 2 * T] = en
    tpv = np.ascontiguousarray(vals.reshape(N_KC, 128).T)      # [128, N_KC]

    cms = []
    for c in range(NCORES):
        tgc = tg[c * BS:(c + 1) * BS]
        cnt = np.zeros((BS, nent), np.float32)
        eidx = tgc[:, :-1] * T + tgc[:, 1:]
        np.add.at(cnt, (np.repeat(np.arange(BS), S - 1), eidx.reshape(-1)), 1.0)
        cnt[np.arange(BS), T * T + tgc[:, 0]] += 1.0
        cnt[np.arange(BS), T * T + T + tgc[:, -1]] += 1.0
        cm = cnt.reshape(BS, N_KC, 128).transpose(2, 1, 0)     # [128, N_KC, BS]
        cms.append(np.ascontiguousarray(cm).reshape(128, N_KC * BS))

    eye = np.eye(BS, dtype=np.float32)
    on2 = np.ones((2, 1), np.float32)
    on48 = np.ones((T, 1), np.float32)

    in_maps = []
    for c in range(NCORES):
        in_maps.append({
            "emp": emps[c],
            "ge": ges[c],
            "bdw": bdw,
            "trT": trT,
            "ser": ser_rep,
            "csm": csm,
            "cm": cms[c],
            "tp": tpv,
            "eye": eye,
            "on2": on2,
            "on48": on48,
        })
    return in_maps


def kernel(emissions, tags, mask, transitions, start_transitions,
           end_transitions):
    global LAST_RESULTS
    in_maps = _marshal(emissions, tags, transitions, start_transitions,
                       end_transitions)
    nc = _get_module()
    res = run_bass_kernel_spmd(
        nc, in_maps, core_ids=list(range(NCORES)),
        trace=bool(os.environ.get("CRF_TRACE")),
    )
    LAST_RESULTS = res
    out = np.concatenate([res.results[c]["res"].reshape(BS)
                          for c in range(NCORES)])
    return out.astype(np.float32)
